# revision 1
# baseline (speedup 1.0000x reference)
"""BinaryLinear TRN2 kernel: out = x @ (sign(W) * alpha).T + bias.

Shapes (hardcoded): x [8192, 4096] f32, W [4096, 4096] f32,
alpha [4096, 1] f32, bias [4096] f32 -> out [8192, 4096] f32.

Strategy: column-parallel over 8 NeuronCores (each core owns 512
out_features).  Per core the weight shard is binarized on-device with the
Sign activation into float32r (11-bit-mantissa fp32; sign values +-1 are
exact) and kept resident in SBUF.  x.T is streamed in 128-column chunks
and split on the fly into hi = f32r(x), lo = f32r(x - hi); two f32r
matmul passes accumulate hi+lo into the same PSUM bank, which yields
fp32-class accuracy (~3e-7 max rel) while each f32r matmul runs at full
bf16 PE rate (~216 ns per 128x128x512 MM).  alpha/bias are applied on
the output tile with two DVE ops against partition-broadcast tiles.
"""

import numpy as np

import concourse.bass as bass
import concourse.tile as tile
from concourse import bacc
import concourse.mybir as mybir
from concourse.bass_utils import run_bass_kernel_spmd

F32 = mybir.dt.float32
F32R = mybir.dt.float32r
ALU = mybir.AluOpType

B, IN, OUT = 8192, 4096, 4096
NCORES = 8
OSH = OUT // NCORES          # 512 out_features per core
KT = IN // 128               # 32 contraction tiles
BT = B // 128                # 64 batch tiles per core

# two f32r passes (hi+lo) -> fp32-class accuracy; "r1" = single pass
MODE = "r2"

_CACHE = {}


def _build(mode=MODE):
    nc = bacc.Bacc("TRN2", target_bir_lowering=False, debug=False)
    xT_d = nc.dram_tensor("xT", [IN, B], F32, kind="ExternalInput").ap()
    wT_d = nc.dram_tensor("wT", [IN, OSH], F32, kind="ExternalInput").ap()
    alpha_d = nc.dram_tensor("alpha", [OSH], F32, kind="ExternalInput").ap()
    bias_d = nc.dram_tensor("bias", [OSH], F32, kind="ExternalInput").ap()
    out_d = nc.dram_tensor("out", [B, OSH], F32, kind="ExternalOutput").ap()

    with tile.TileContext(nc) as tc:
        with (
            tc.tile_pool(name="const", bufs=1) as const,
            tc.tile_pool(name="wstage", bufs=2) as wstage,
            tc.tile_pool(name="xpool", bufs=2) as xpool,
            tc.tile_pool(name="hpool", bufs=2) as hpool,
            tc.tile_pool(name="lpool", bufs=2) as lpool,
            tc.tile_pool(name="opool", bufs=3) as opool,
            tc.tile_pool(name="ps", bufs=8, space="PSUM") as ps,
        ):
            alpha_b = const.tile([128, OSH], F32, name="alpha_b")
            nc.sync.dma_start(alpha_b[:], alpha_d.partition_broadcast(128))
            bias_b = const.tile([128, OSH], F32, name="bias_b")
            nc.sync.dma_start(bias_b[:], bias_d.partition_broadcast(128))

            # resident binarized weight shard: [128, KT, OSH] f32r
            w_r = const.tile([128, KT, OSH], F32R, name="w_r")
            wT_t = wT_d.rearrange("(it p) o -> p it o", p=128)
            for it in range(KT):
                w_f = wstage.tile([128, OSH], F32, tag="w_f", name="w_f")
                nc.sync.dma_start(w_f[:], wT_t[:, it, :])
                nc.scalar.sign(w_r[:, it, :], w_f[:])

            xT_t = xT_d.rearrange("(it p) b -> p it b", p=128)
            for bt in range(BT):
                bs = bass.ts(bt, 128)
                x_f = xpool.tile([128, KT, 128], F32, tag="x_f", name="x_f")
                nc.sync.dma_start(x_f[:], xT_t[:, :, bs])
                x_h = hpool.tile([128, KT, 128], F32R, tag="x_h", name="x_h")
                nc.scalar.copy(x_h[:], x_f[:])
                if mode == "r2":
                    x_l = lpool.tile([128, KT, 128], F32R, tag="x_l", name="x_l")
                    nc.vector.scalar_tensor_tensor(
                        x_l[:], x_f[:], 0.0, x_h[:], ALU.bypass, ALU.subtract
                    )
                p = ps.tile([128, OSH], F32, tag="p", name="p")
                for it in range(KT):
                    nc.tensor.matmul(
                        p[:], x_h[:, it, :], w_r[:, it, :],
                        start=(it == 0),
                        stop=(mode == "r1" and it == KT - 1),
                    )
                    if mode == "r2":
                        nc.tensor.matmul(
                            p[:], x_l[:, it, :], w_r[:, it, :],
                            start=False, stop=(it == KT - 1),
                        )
                t = opool.tile([128, OSH], F32, tag="t", name="t")
                nc.vector.tensor_mul(t[:], p[:], alpha_b[:])
                o = opool.tile([128, OSH], F32, tag="o", name="o")
                nc.vector.tensor_add(o[:], t[:], bias_b[:])
                nc.sync.dma_start(out_d[bs, :], o[:])

    nc.compile()
    return nc


def kernel(x, weight_fp, alpha, bias):
    x = np.asarray(x, dtype=np.float32)
    weight_fp = np.asarray(weight_fp, dtype=np.float32)
    alpha = np.asarray(alpha, dtype=np.float32).reshape(-1)
    bias = np.asarray(bias, dtype=np.float32).reshape(-1)
    assert x.shape == (B, IN) and weight_fp.shape == (OUT, IN)

    if "nc" not in _CACHE:
        _CACHE["nc"] = _build()
    nc = _CACHE["nc"]

    xT = np.ascontiguousarray(x.T)               # [IN, B]
    wT = np.ascontiguousarray(weight_fp.T)       # [IN, OUT]
    in_maps = []
    for c in range(NCORES):
        sl = slice(c * OSH, (c + 1) * OSH)
        in_maps.append({
            "xT": xT,
            "wT": np.ascontiguousarray(wT[:, sl]),
            "alpha": np.ascontiguousarray(alpha[sl]),
            "bias": np.ascontiguousarray(bias[sl]),
        })
    res = run_bass_kernel_spmd(nc, in_maps, list(range(NCORES)))
    out = np.concatenate(
        [res.results[c]["out"] for c in range(NCORES)], axis=1
    )
    return np.ascontiguousarray(out, dtype=np.float32)


# revision 11
# speedup vs baseline: 1.0926x; 1.0926x over previous
"""BinaryLinear TRN2 kernel: out = x @ (sign(W) * alpha).T + bias.

Shapes (hardcoded): x [8192, 4096] f32, W [4096, 4096] f32,
alpha [4096, 1] f32, bias [4096] f32 -> out [8192, 4096] f32.

Strategy: column-parallel over 8 NeuronCores (each core owns 512
out_features).  Per core the weight shard is binarized on-device with
the Sign activation (sign values +-1 are exact in fp16) and kept
resident in SBUF.  x.T is streamed in 128-column chunks (host pre-tiles
it so each chunk is a single contiguous 2 MB block) and split on the fly
into hi = f16(4096*x) and lo = f16(4096*x - hi); the power-of-two scale
is exact and keeps the low term inside fp16's normal range.  Both f16
matmul passes accumulate into the same PSUM bank (the common scale is
divided out with alpha afterwards), which yields fp32-class accuracy
(~3e-7 max rel) while each f16 matmul runs at full PE rate (~216 ns per
128x128x512 MM, weight loads hidden by FWL).  alpha/bias are applied on
the output tile with two DVE ops against partition-broadcast tiles.
"""

import numpy as np

import concourse.bass as bass
import concourse.tile as tile
from concourse import bacc
import concourse.mybir as mybir
from concourse.bass_utils import run_bass_kernel_spmd

F32 = mybir.dt.float32
F32R = mybir.dt.float32r
F16 = mybir.dt.float16
ALU = mybir.AluOpType

B, IN, OUT = 8192, 4096, 4096
NCORES = 8
OSH = OUT // NCORES          # 512 out_features per core
KT = IN // 128               # 32 contraction tiles
BT = B // 128                # 64 batch tiles per core
SC = 4096.0                  # lo-term scale for the f16 mode

MODE = "f16"                 # "f16" | "r2" | "r1"

_CACHE = {}


def _build(mode=MODE):
    wdt = F16 if mode == "f16" else F32R
    nc = bacc.Bacc("TRN2", target_bir_lowering=False, debug=False)
    # x pre-tiled on host: xT[bt, p, it, b] = x[bt*128 + b, it*128 + p]
    xt_d = nc.dram_tensor("xT", [BT, 128, KT, 128], F32, kind="ExternalInput").ap()
    wT_d = nc.dram_tensor("wT", [IN, OSH], F32, kind="ExternalInput").ap()
    alpha_d = nc.dram_tensor("alpha", [OSH], F32, kind="ExternalInput").ap()
    bias_d = nc.dram_tensor("bias", [OSH], F32, kind="ExternalInput").ap()
    out_d = nc.dram_tensor("out", [B, OSH], F32, kind="ExternalOutput").ap()

    with tile.TileContext(nc) as tc:
        with (
            tc.tile_pool(name="const", bufs=1) as const,
            tc.tile_pool(name="wstage", bufs=2) as wstage,
            tc.tile_pool(name="xpool", bufs=3) as xpool,
            tc.tile_pool(name="hpool", bufs=4) as hpool,
            tc.tile_pool(name="lpool", bufs=4) as lpool,
            tc.tile_pool(name="opool", bufs=3) as opool,
            tc.tile_pool(name="ps", bufs=8, space="PSUM") as ps,
        ):
            def load_chunk(bt):
                x_f = xpool.tile([128, KT, 128], F32, tag="x_f", name="x_f")
                nc.sync.dma_start(x_f[:], xt_d[bt])
                x_h = hpool.tile([128, KT, 128], wdt, tag="x_h", name="x_h")
                if mode == "f16":
                    # hi = f16(SC*x) (power-of-two scale, exact)
                    nc.scalar.mul(x_h[:], x_f[:], SC)
                else:
                    nc.scalar.copy(x_h[:], x_f[:])
                x_l = None
                if mode != "r1":
                    x_l = lpool.tile([128, KT, 128], wdt, tag="x_l",
                                     name="x_l")
                    if mode == "f16":
                        # lo = f16(SC*x - hi)
                        nc.vector.scalar_tensor_tensor(
                            x_l[:], x_f[:], SC, x_h[:],
                            ALU.mult, ALU.subtract)
                    else:
                        nc.vector.scalar_tensor_tensor(
                            x_l[:], x_f[:], 0.0, x_h[:],
                            ALU.bypass, ALU.subtract)
                return x_h, x_l

            # x chunk 0 first so its DMA heads the queue
            pending = [load_chunk(0)]

            alpha_b = const.tile([128, OSH], F32, name="alpha_b")
            nc.sync.dma_start(alpha_b[:], alpha_d.partition_broadcast(128))
            bias_b = const.tile([128, OSH], F32, name="bias_b")
            nc.sync.dma_start(bias_b[:], bias_d.partition_broadcast(128))
            if mode == "f16":
                alpha_eff = const.tile([128, OSH], F32, name="alpha_eff")
                nc.vector.tensor_scalar_mul(alpha_eff[:], alpha_b[:], 1.0 / SC)
            else:
                alpha_eff = alpha_b

            # resident binarized weight shard, one tile per k-tile
            wT_t = wT_d.rearrange("(it p) o -> p it o", p=128)
            w_r = []
            for it in range(KT):
                w_f = wstage.tile([128, OSH], F32, tag="w_f", name="w_f")
                nc.sync.dma_start(w_f[:], wT_t[:, it, :])
                w_rt = const.tile([128, OSH], wdt, name=f"w_r{it}")
                nc.scalar.sign(w_rt[:], w_f[:])
                w_r.append(w_rt)

            for bt in range(BT):
                x_h, x_l = pending.pop(0)
                if bt + 1 < BT:
                    pending.append(load_chunk(bt + 1))
                p = ps.tile([128, OSH], F32, tag="p", name="p")
                for it in range(KT):
                    nc.tensor.matmul(
                        p[:], x_h[:, it, :], w_r[it][:],
                        start=(it == 0),
                        stop=(mode == "r1" and it == KT - 1))
                    if mode != "r1":
                        nc.tensor.matmul(
                            p[:], x_l[:, it, :], w_r[it][:],
                            start=False, stop=(it == KT - 1))
                # out = p * alpha_eff + bias  (alpha_eff = alpha/SC for f16)
                t = opool.tile([128, OSH], F32, tag="t", name="t")
                nc.vector.scalar_tensor_tensor(
                    t[:], p[:], 0.0, alpha_eff[:], ALU.bypass, ALU.mult)
                o = opool.tile([128, OSH], F32, tag="o", name="o")
                nc.vector.tensor_add(o[:], t[:], bias_b[:])
                nc.sync.dma_start(out_d[bass.ts(bt, 128), :], o[:])

    nc.compile()
    return nc


def _prep_inputs(x, weight_fp, alpha, bias):
    x = np.asarray(x, dtype=np.float32)
    weight_fp = np.asarray(weight_fp, dtype=np.float32)
    alpha = np.asarray(alpha, dtype=np.float32).reshape(-1)
    bias = np.asarray(bias, dtype=np.float32).reshape(-1)
    assert x.shape == (B, IN) and weight_fp.shape == (OUT, IN)

    # [bt, p, it, b] <- x[bt*128+b, it*128+p]
    xT = np.ascontiguousarray(
        x.reshape(BT, 128, KT, 128).transpose(0, 3, 2, 1)
    )
    in_maps = []
    for c in range(NCORES):
        sl = slice(c * OSH, (c + 1) * OSH)
        in_maps.append({
            "xT": xT,
            "wT": np.ascontiguousarray(weight_fp[sl].T),
            "alpha": np.ascontiguousarray(alpha[sl]),
            "bias": np.ascontiguousarray(bias[sl]),
        })
    return in_maps


def kernel(x, weight_fp, alpha, bias):
    if "nc" not in _CACHE:
        _CACHE["nc"] = _build()
    nc = _CACHE["nc"]
    in_maps = _prep_inputs(x, weight_fp, alpha, bias)
    res = run_bass_kernel_spmd(nc, in_maps, list(range(NCORES)))
    out = np.concatenate(
        [res.results[c]["out"] for c in range(NCORES)], axis=1
    )
    return np.ascontiguousarray(out, dtype=np.float32)


# revision 16
# speedup vs baseline: 1.0972x; 1.0041x over previous
"""BinaryLinear TRN2 kernel: out = x @ (sign(W) * alpha).T + bias.

Shapes (hardcoded): x [8192, 4096] f32, W [4096, 4096] f32,
alpha [4096, 1] f32, bias [4096] f32 -> out [8192, 4096] f32.

Strategy: column-parallel over 8 NeuronCores (each core owns 512
out_features).  Per core the weight shard is binarized on-device with
the Sign activation (sign values +-1 are exact in fp16) and kept
resident in SBUF.  x.T is streamed in 128-column chunks (host pre-tiles
it so each chunk is a single contiguous 2 MB block) and split on the fly
into hi = f16(4096*x) and lo = f16(4096*x - hi); the power-of-two scale
is exact and keeps the low term inside fp16's normal range.  Both f16
matmul passes accumulate into the same PSUM bank (the common scale is
divided out with alpha afterwards), which yields fp32-class accuracy
(~3e-7 max rel) while each f16 matmul runs at full PE rate (~216 ns per
128x128x512 MM, weight loads hidden by FWL).  alpha/bias are applied on
the output tile with two DVE ops against partition-broadcast tiles.
"""

import numpy as np

import concourse.bass as bass
import concourse.tile as tile
from concourse import bacc
import concourse.mybir as mybir
from concourse.bass_utils import run_bass_kernel_spmd

F32 = mybir.dt.float32
F32R = mybir.dt.float32r
F16 = mybir.dt.float16
ALU = mybir.AluOpType

B, IN, OUT = 8192, 4096, 4096
NCORES = 8
OSH = OUT // NCORES          # 512 out_features per core
KT = IN // 128               # 32 contraction tiles
BT = B // 128                # 64 batch tiles per core
SC = 4096.0                  # lo-term scale for the f16 mode

MODE = "f16"                 # "f16" | "r2" | "r1"

_CACHE = {}


def _build(mode=MODE):
    wdt = F16 if mode == "f16" else F32R
    nc = bacc.Bacc("TRN2", target_bir_lowering=False, debug=False)
    # x pre-tiled on host: xT[bt, p, it, b] = x[bt*128 + b, it*128 + p]
    xt_d = nc.dram_tensor("xT", [BT, 128, KT, 128], F32, kind="ExternalInput").ap()
    wT_d = nc.dram_tensor("wT", [IN, OSH], F32, kind="ExternalInput").ap()
    alpha_d = nc.dram_tensor("alpha", [OSH], F32, kind="ExternalInput").ap()
    bias_d = nc.dram_tensor("bias", [OSH], F32, kind="ExternalInput").ap()
    out_d = nc.dram_tensor("out", [B, OSH], F32, kind="ExternalOutput").ap()

    with tile.TileContext(nc) as tc:
        with (
            tc.tile_pool(name="const", bufs=1) as const,
            tc.tile_pool(name="wstage", bufs=2) as wstage,
            tc.tile_pool(name="xpool", bufs=3) as xpool,
            tc.tile_pool(name="hpool", bufs=4) as hpool,
            tc.tile_pool(name="lpool", bufs=4) as lpool,
            tc.tile_pool(name="opool", bufs=3) as opool,
            tc.tile_pool(name="ps", bufs=8, space="PSUM") as ps,
        ):
            def load_chunk(bt):
                x_f = xpool.tile([128, KT, 128], F32, tag="x_f", name="x_f")
                nc.sync.dma_start(x_f[:], xt_d[bt])
                x_h = hpool.tile([128, KT, 128], wdt, tag="x_h", name="x_h")
                if mode == "f16":
                    # hi = f16(SC*x) (power-of-two scale, exact)
                    nc.scalar.mul(x_h[:], x_f[:], SC)
                else:
                    nc.scalar.copy(x_h[:], x_f[:])
                x_l = None
                if mode != "r1":
                    x_l = lpool.tile([128, KT, 128], wdt, tag="x_l",
                                     name="x_l")
                    if mode == "f16":
                        # lo = f16(SC*x - hi)
                        nc.vector.scalar_tensor_tensor(
                            x_l[:], x_f[:], SC, x_h[:],
                            ALU.mult, ALU.subtract)
                    else:
                        nc.vector.scalar_tensor_tensor(
                            x_l[:], x_f[:], 0.0, x_h[:],
                            ALU.bypass, ALU.subtract)
                return x_h, x_l

            # x chunk 0 first so its DMA heads the queue
            pending = [load_chunk(0)]

            alpha_b = const.tile([128, OSH], F32, name="alpha_b")
            nc.sync.dma_start(alpha_b[:], alpha_d.partition_broadcast(128))
            bias_b = const.tile([128, OSH], F32, name="bias_b")
            nc.sync.dma_start(bias_b[:], bias_d.partition_broadcast(128))
            if mode == "f16":
                alpha_eff = const.tile([128, OSH], F32, name="alpha_eff")
                nc.vector.tensor_scalar_mul(alpha_eff[:], alpha_b[:], 1.0 / SC)
            else:
                alpha_eff = alpha_b

            # resident binarized weight shard, one tile per k-tile
            wT_t = wT_d.rearrange("(it p) o -> p it o", p=128)
            w_r = []
            for it in range(KT):
                w_f = wstage.tile([128, OSH], F32, tag="w_f", name="w_f")
                nc.sync.dma_start(w_f[:], wT_t[:, it, :])
                w_rt = const.tile([128, OSH], wdt, name=f"w_r{it}")
                nc.scalar.sign(w_rt[:], w_f[:])
                w_r.append(w_rt)

            for bt in range(BT):
                x_h, x_l = pending.pop(0)
                if bt + 1 < BT:
                    pending.append(load_chunk(bt + 1))
                p = ps.tile([128, OSH], F32, tag="p", name="p")
                for it in range(KT):
                    nc.tensor.matmul(
                        p[:], x_h[:, it, :], w_r[it][:],
                        start=(it == 0),
                        stop=(mode == "r1" and it == KT - 1))
                    if mode != "r1":
                        nc.tensor.matmul(
                            p[:], x_l[:, it, :], w_r[it][:],
                            start=False, stop=(it == KT - 1))
                # out = p * alpha_eff + bias  (alpha_eff = alpha/SC for f16)
                t = opool.tile([128, OSH], F32, tag="t", name="t")
                nc.vector.scalar_tensor_tensor(
                    t[:], p[:], 0.0, alpha_eff[:], ALU.bypass, ALU.mult)
                o = opool.tile([128, OSH], F32, tag="o", name="o")
                nc.vector.tensor_add(o[:], t[:], bias_b[:])
                nc.sync.dma_start(out_d[bass.ts(bt, 128), :], o[:])

    nc.compile()
    return nc


def _prep_inputs(x, weight_fp, alpha, bias):
    x = np.asarray(x, dtype=np.float32)
    weight_fp = np.asarray(weight_fp, dtype=np.float32)
    alpha = np.asarray(alpha, dtype=np.float32).reshape(-1)
    bias = np.asarray(bias, dtype=np.float32).reshape(-1)
    assert x.shape == (B, IN) and weight_fp.shape == (OUT, IN)

    # [bt, p, it, b] <- x[bt*128+b, it*128+p]
    xT = np.ascontiguousarray(
        x.reshape(BT, 128, KT, 128).transpose(0, 3, 2, 1)
    )
    in_maps = []
    for c in range(NCORES):
        sl = slice(c * OSH, (c + 1) * OSH)
        in_maps.append({
            "xT": xT,
            "wT": np.ascontiguousarray(weight_fp[sl].T),
            "alpha": np.ascontiguousarray(alpha[sl]),
            "bias": np.ascontiguousarray(bias[sl]),
        })
    return in_maps


def kernel(x, weight_fp, alpha, bias):
    if "nc" not in _CACHE:
        _CACHE["nc"] = _build()
    nc = _CACHE["nc"]
    in_maps = _prep_inputs(x, weight_fp, alpha, bias)
    res = run_bass_kernel_spmd(nc, in_maps, list(range(NCORES)))
    out = np.concatenate(
        [res.results[c]["out"] for c in range(NCORES)], axis=1
    )
    return np.ascontiguousarray(out, dtype=np.float32)


# revision 17
# speedup vs baseline: 1.1061x; 1.0081x over previous
"""BinaryLinear TRN2 kernel: out = x @ (sign(W) * alpha).T + bias.

Shapes (hardcoded): x [8192, 4096] f32, W [4096, 4096] f32,
alpha [4096, 1] f32, bias [4096] f32 -> out [8192, 4096] f32.

Strategy: column-parallel over 8 NeuronCores (each core owns 512
out_features).  Per core the weight shard is binarized on-device with
the Sign activation (sign values +-1 are exact in fp16) and kept
resident in SBUF.  x.T is streamed in 128-column chunks (host pre-tiles
it so each chunk is a single contiguous 2 MB block) and split on the fly
into hi = f16(4096*x) and lo = f16(4096*x - hi); the power-of-two scale
is exact and keeps the low term inside fp16's normal range.  Both f16
matmul passes accumulate into the same PSUM bank (the common scale is
divided out with alpha afterwards), which yields fp32-class accuracy
(~3e-7 max rel) while each f16 matmul runs at full PE rate (~216 ns per
128x128x512 MM, weight loads hidden by FWL).  alpha/bias are applied on
the output tile with two DVE ops against partition-broadcast tiles.
"""

import numpy as np

import concourse.bass as bass
import concourse.tile as tile
from concourse import bacc
import concourse.mybir as mybir
from concourse.bass_utils import run_bass_kernel_spmd

F32 = mybir.dt.float32
F32R = mybir.dt.float32r
F16 = mybir.dt.float16
ALU = mybir.AluOpType

B, IN, OUT = 8192, 4096, 4096
NCORES = 8
OSH = OUT // NCORES          # 512 out_features per core
KT = IN // 128               # 32 contraction tiles
BT = B // 128                # 64 batch tiles per core
SC = 4096.0                  # lo-term scale for the f16 mode

MODE = "f16"                 # "f16" | "r2" | "r1"

_CACHE = {}


def _build(mode=MODE):
    wdt = F16 if mode == "f16" else F32R
    nc = bacc.Bacc("TRN2", target_bir_lowering=False, debug=False)
    # x pre-tiled on host: xT[bt, p, it, b] = x[bt*128 + b, it*128 + p]
    xt_d = nc.dram_tensor("xT", [BT, 128, KT, 128], F32, kind="ExternalInput").ap()
    wT_d = nc.dram_tensor("wT", [IN, OSH], F32, kind="ExternalInput").ap()
    alpha_d = nc.dram_tensor("alpha", [OSH], F32, kind="ExternalInput").ap()
    bias_d = nc.dram_tensor("bias", [OSH], F32, kind="ExternalInput").ap()
    out_d = nc.dram_tensor("out", [B, OSH], F32, kind="ExternalOutput").ap()

    with tile.TileContext(nc) as tc:
        with (
            tc.tile_pool(name="const", bufs=1) as const,
            tc.tile_pool(name="wstage", bufs=2) as wstage,
            tc.tile_pool(name="xpool", bufs=2) as xpool,
            tc.tile_pool(name="hpool", bufs=6) as hpool,
            tc.tile_pool(name="lpool", bufs=6) as lpool,
            tc.tile_pool(name="opool", bufs=3) as opool,
            tc.tile_pool(name="ps", bufs=8, space="PSUM") as ps,
        ):
            def load_chunk(bt):
                x_f = xpool.tile([128, KT, 128], F32, tag="x_f", name="x_f")
                nc.sync.dma_start(x_f[:], xt_d[bt])
                x_h = hpool.tile([128, KT, 128], wdt, tag="x_h", name="x_h")
                if mode == "f16":
                    # hi = f16(SC*x) (power-of-two scale, exact)
                    nc.scalar.mul(x_h[:], x_f[:], SC)
                else:
                    nc.scalar.copy(x_h[:], x_f[:])
                x_l = None
                if mode != "r1":
                    x_l = lpool.tile([128, KT, 128], wdt, tag="x_l",
                                     name="x_l")
                    if mode == "f16":
                        # lo = f16(SC*x - hi)
                        nc.vector.scalar_tensor_tensor(
                            x_l[:], x_f[:], SC, x_h[:],
                            ALU.mult, ALU.subtract)
                    else:
                        nc.vector.scalar_tensor_tensor(
                            x_l[:], x_f[:], 0.0, x_h[:],
                            ALU.bypass, ALU.subtract)
                return x_h, x_l

            # batch tiles processed in groups of G with the contraction loop
            # outermost: each weight k-tile feeds 2*G matmuls the moment it
            # arrives, so the W DMA stream never starves the PE during ramp-in
            G = 3
            groups = [list(range(g, min(g + G, BT))) for g in range(0, BT, G)]
            chunks = {}
            # group-0 x chunks interleaved with the W stream on the DMA queue
            chunks[groups[0][0]] = load_chunk(groups[0][0])

            alpha_b = const.tile([128, OSH], F32, name="alpha_b")
            nc.sync.dma_start(alpha_b[:], alpha_d.partition_broadcast(128))
            bias_b = const.tile([128, OSH], F32, name="bias_b")
            nc.sync.dma_start(bias_b[:], bias_d.partition_broadcast(128))
            if mode == "f16":
                alpha_eff = const.tile([128, OSH], F32, name="alpha_eff")
                nc.vector.tensor_scalar_mul(alpha_eff[:], alpha_b[:], 1.0 / SC)
            else:
                alpha_eff = alpha_b

            # resident binarized weight shard, one tile per k-tile
            wT_t = wT_d.rearrange("(it p) o -> p it o", p=128)
            w_r = []
            for it in range(KT):
                if it == 8 and len(groups[0]) > 1:
                    chunks[groups[0][1]] = load_chunk(groups[0][1])
                if it == 16 and len(groups[0]) > 2:
                    chunks[groups[0][2]] = load_chunk(groups[0][2])
                w_f = wstage.tile([128, OSH], F32, tag="w_f", name="w_f")
                nc.sync.dma_start(w_f[:], wT_t[:, it, :])
                w_rt = const.tile([128, OSH], wdt, name=f"w_r{it}")
                nc.scalar.sign(w_rt[:], w_f[:])
                w_r.append(w_rt)

            for gi, grp in enumerate(groups):
                pt = {b: ps.tile([128, OSH], F32, tag="p", name=f"p{b}")
                      for b in grp}
                nxt = groups[gi + 1] if gi + 1 < len(groups) else []
                load_at = {(j + 1) * KT // (len(nxt) + 1): nxt[j]
                           for j in range(len(nxt))}
                for it in range(KT):
                    if it in load_at:
                        chunks[load_at[it]] = load_chunk(load_at[it])
                    for b in grp:
                        x_h, x_l = chunks[b]
                        nc.tensor.matmul(
                            pt[b][:], x_h[:, it, :], w_r[it][:],
                            start=(it == 0),
                            stop=(mode == "r1" and it == KT - 1))
                        if mode != "r1":
                            nc.tensor.matmul(
                                pt[b][:], x_l[:, it, :], w_r[it][:],
                                start=False, stop=(it == KT - 1))
                for b in grp:
                    del chunks[b]
                    # out = p * alpha_eff + bias (alpha_eff = alpha/SC for f16)
                    t = opool.tile([128, OSH], F32, tag="t", name="t")
                    nc.vector.scalar_tensor_tensor(
                        t[:], pt[b][:], 0.0, alpha_eff[:],
                        ALU.bypass, ALU.mult)
                    o = opool.tile([128, OSH], F32, tag="o", name="o")
                    nc.vector.tensor_add(o[:], t[:], bias_b[:])
                    nc.sync.dma_start(out_d[bass.ts(b, 128), :], o[:])

    nc.compile()
    return nc


def _prep_inputs(x, weight_fp, alpha, bias):
    x = np.asarray(x, dtype=np.float32)
    weight_fp = np.asarray(weight_fp, dtype=np.float32)
    alpha = np.asarray(alpha, dtype=np.float32).reshape(-1)
    bias = np.asarray(bias, dtype=np.float32).reshape(-1)
    assert x.shape == (B, IN) and weight_fp.shape == (OUT, IN)

    # [bt, p, it, b] <- x[bt*128+b, it*128+p]
    xT = np.ascontiguousarray(
        x.reshape(BT, 128, KT, 128).transpose(0, 3, 2, 1)
    )
    in_maps = []
    for c in range(NCORES):
        sl = slice(c * OSH, (c + 1) * OSH)
        in_maps.append({
            "xT": xT,
            "wT": np.ascontiguousarray(weight_fp[sl].T),
            "alpha": np.ascontiguousarray(alpha[sl]),
            "bias": np.ascontiguousarray(bias[sl]),
        })
    return in_maps


def kernel(x, weight_fp, alpha, bias):
    if "nc" not in _CACHE:
        _CACHE["nc"] = _build()
    nc = _CACHE["nc"]
    in_maps = _prep_inputs(x, weight_fp, alpha, bias)
    res = run_bass_kernel_spmd(nc, in_maps, list(range(NCORES)))
    out = np.concatenate(
        [res.results[c]["out"] for c in range(NCORES)], axis=1
    )
    return np.ascontiguousarray(out, dtype=np.float32)


# revision 18
# speedup vs baseline: 1.1205x; 1.0130x over previous
"""BinaryLinear TRN2 kernel: out = x @ (sign(W) * alpha).T + bias.

Shapes (hardcoded): x [8192, 4096] f32, W [4096, 4096] f32,
alpha [4096, 1] f32, bias [4096] f32 -> out [8192, 4096] f32.

Strategy: column-parallel over 8 NeuronCores (each core owns 512
out_features).  Per core the weight shard is binarized on-device with
the Sign activation (sign values +-1 are exact in fp16) and kept
resident in SBUF.  x.T is streamed in 128-column chunks (host pre-tiles
it so each chunk is a single contiguous 2 MB block) and split on the fly
into hi = f16(4096*x) and lo = f16(4096*x - hi); the power-of-two scale
is exact and keeps the low term inside fp16's normal range.  Both f16
matmul passes accumulate into the same PSUM bank (the common scale is
divided out with alpha afterwards), which yields fp32-class accuracy
(~3e-7 max rel) while each f16 matmul runs at full PE rate (~216 ns per
128x128x512 MM, weight loads hidden by FWL).  alpha/bias are applied on
the output tile with two DVE ops against partition-broadcast tiles.
"""

import numpy as np

import concourse.bass as bass
import concourse.tile as tile
from concourse import bacc
import concourse.mybir as mybir
from concourse.bass_utils import run_bass_kernel_spmd

F32 = mybir.dt.float32
F32R = mybir.dt.float32r
F16 = mybir.dt.float16
ALU = mybir.AluOpType

B, IN, OUT = 8192, 4096, 4096
NCORES = 8
OSH = OUT // NCORES          # 512 out_features per core
KT = IN // 128               # 32 contraction tiles
BT = B // 128                # 64 batch tiles per core
SC = 4096.0                  # lo-term scale for the f16 mode

MODE = "f16"                 # "f16" | "r2" | "r1"

_CACHE = {}


def _build(mode=MODE):
    wdt = F16 if mode == "f16" else F32R
    nc = bacc.Bacc("TRN2", target_bir_lowering=False, debug=False)
    # x pre-tiled on host: xT[bt, p, it, b] = x[bt*128 + b, it*128 + p]
    xt_d = nc.dram_tensor("xT", [BT, 128, KT, 128], F32, kind="ExternalInput").ap()
    wT_d = nc.dram_tensor("wT", [IN, OSH], F32, kind="ExternalInput").ap()
    alpha_d = nc.dram_tensor("alpha", [OSH], F32, kind="ExternalInput").ap()
    bias_d = nc.dram_tensor("bias", [OSH], F32, kind="ExternalInput").ap()
    out_d = nc.dram_tensor("out", [B, OSH], F32, kind="ExternalOutput").ap()

    with tile.TileContext(nc) as tc:
        with (
            tc.tile_pool(name="const", bufs=1) as const,
            tc.tile_pool(name="wstage", bufs=3) as wstage,
            tc.tile_pool(name="xpool", bufs=2) as xpool,
            tc.tile_pool(name="hpool", bufs=6) as hpool,
            tc.tile_pool(name="lpool", bufs=6) as lpool,
            tc.tile_pool(name="opool", bufs=4) as opool,
            tc.tile_pool(name="ps", bufs=8, space="PSUM") as ps,
        ):
            def load_chunk(bt):
                x_f = xpool.tile([128, KT, 128], F32, tag="x_f", name="x_f")
                nc.sync.dma_start(x_f[:], xt_d[bt])
                x_h = hpool.tile([128, KT, 128], wdt, tag="x_h", name="x_h")
                if mode == "f16":
                    # hi = f16(SC*x) (power-of-two scale, exact)
                    nc.scalar.mul(x_h[:], x_f[:], SC)
                else:
                    nc.scalar.copy(x_h[:], x_f[:])
                x_l = None
                if mode != "r1":
                    x_l = lpool.tile([128, KT, 128], wdt, tag="x_l",
                                     name="x_l")
                    if mode == "f16":
                        # lo = f16(SC*x - hi)
                        nc.vector.scalar_tensor_tensor(
                            x_l[:], x_f[:], SC, x_h[:],
                            ALU.mult, ALU.subtract)
                    else:
                        nc.vector.scalar_tensor_tensor(
                            x_l[:], x_f[:], 0.0, x_h[:],
                            ALU.bypass, ALU.subtract)
                return x_h, x_l

            # batch tiles processed in groups of G with the contraction loop
            # outermost: each weight k-tile feeds 2*G matmuls the moment it
            # arrives, so the W DMA stream never starves the PE during ramp-in
            G = 3
            groups = [list(range(g, min(g + G, BT))) for g in range(0, BT, G)]
            chunks = {}
            # group-0 x chunks interleaved with the W stream on the DMA queue
            chunks[groups[0][0]] = load_chunk(groups[0][0])

            alpha_b = const.tile([128, OSH], F32, name="alpha_b")
            nc.sync.dma_start(alpha_b[:], alpha_d.partition_broadcast(128))
            bias_b = const.tile([128, OSH], F32, name="bias_b")
            nc.sync.dma_start(bias_b[:], bias_d.partition_broadcast(128))
            if mode == "f16":
                alpha_eff = const.tile([128, OSH], F32, name="alpha_eff")
                nc.vector.tensor_scalar_mul(alpha_eff[:], alpha_b[:], 1.0 / SC)
            else:
                alpha_eff = alpha_b

            # resident binarized weight shard, one tile per k-tile
            wT_t = wT_d.rearrange("(it p) o -> p it o", p=128)
            w_r = []
            for it in range(KT):
                if it == 8 and len(groups[0]) > 1:
                    chunks[groups[0][1]] = load_chunk(groups[0][1])
                if it == 16 and len(groups[0]) > 2:
                    chunks[groups[0][2]] = load_chunk(groups[0][2])
                w_f = wstage.tile([128, OSH], F32, tag="w_f", name="w_f")
                nc.sync.dma_start(w_f[:], wT_t[:, it, :])
                w_rt = const.tile([128, OSH], wdt, name=f"w_r{it}")
                nc.scalar.sign(w_rt[:], w_f[:])
                w_r.append(w_rt)

            for gi, grp in enumerate(groups):
                pt = {b: ps.tile([128, OSH], F32, tag="p", name=f"p{b}")
                      for b in grp}
                nxt = groups[gi + 1] if gi + 1 < len(groups) else []
                load_at = {(j + 1) * KT // (len(nxt) + 1): nxt[j]
                           for j in range(len(nxt))}
                for it in range(KT):
                    if it in load_at:
                        chunks[load_at[it]] = load_chunk(load_at[it])
                    for b in grp:
                        x_h, x_l = chunks[b]
                        nc.tensor.matmul(
                            pt[b][:], x_h[:, it, :], w_r[it][:],
                            start=(it == 0),
                            stop=(mode == "r1" and it == KT - 1))
                        if mode != "r1":
                            nc.tensor.matmul(
                                pt[b][:], x_l[:, it, :], w_r[it][:],
                                start=False, stop=(it == KT - 1))
                for b in grp:
                    del chunks[b]
                    # out = p * alpha_eff + bias (alpha_eff = alpha/SC for f16)
                    t = opool.tile([128, OSH], F32, tag="t", name="t")
                    nc.vector.scalar_tensor_tensor(
                        t[:], pt[b][:], 0.0, alpha_eff[:],
                        ALU.bypass, ALU.mult)
                    o = opool.tile([128, OSH], F32, tag="o", name="o")
                    nc.vector.tensor_add(o[:], t[:], bias_b[:])
                    nc.sync.dma_start(out_d[bass.ts(b, 128), :], o[:])

    nc.compile()
    return nc


def _prep_inputs(x, weight_fp, alpha, bias):
    x = np.asarray(x, dtype=np.float32)
    weight_fp = np.asarray(weight_fp, dtype=np.float32)
    alpha = np.asarray(alpha, dtype=np.float32).reshape(-1)
    bias = np.asarray(bias, dtype=np.float32).reshape(-1)
    assert x.shape == (B, IN) and weight_fp.shape == (OUT, IN)

    # [bt, p, it, b] <- x[bt*128+b, it*128+p]
    xT = np.ascontiguousarray(
        x.reshape(BT, 128, KT, 128).transpose(0, 3, 2, 1)
    )
    in_maps = []
    for c in range(NCORES):
        sl = slice(c * OSH, (c + 1) * OSH)
        in_maps.append({
            "xT": xT,
            "wT": np.ascontiguousarray(weight_fp[sl].T),
            "alpha": np.ascontiguousarray(alpha[sl]),
            "bias": np.ascontiguousarray(bias[sl]),
        })
    return in_maps


def kernel(x, weight_fp, alpha, bias):
    if "nc" not in _CACHE:
        _CACHE["nc"] = _build()
    nc = _CACHE["nc"]
    in_maps = _prep_inputs(x, weight_fp, alpha, bias)
    res = run_bass_kernel_spmd(nc, in_maps, list(range(NCORES)))
    out = np.concatenate(
        [res.results[c]["out"] for c in range(NCORES)], axis=1
    )
    return np.ascontiguousarray(out, dtype=np.float32)


# revision 19
# speedup vs baseline: 1.1264x; 1.0053x over previous
"""BinaryLinear TRN2 kernel: out = x @ (sign(W) * alpha).T + bias.

Shapes (hardcoded): x [8192, 4096] f32, W [4096, 4096] f32,
alpha [4096, 1] f32, bias [4096] f32 -> out [8192, 4096] f32.

Strategy: column-parallel over 8 NeuronCores (each core owns 512
out_features).  Per core the weight shard is binarized on-device with
the Sign activation (sign values +-1 are exact in fp16) and kept
resident in SBUF.  x.T is streamed in 128-column chunks (host pre-tiles
it so each chunk is a single contiguous 2 MB block) and split on the fly
into hi = f16(4096*x) and lo = f16(4096*x - hi); the power-of-two scale
is exact and keeps the low term inside fp16's normal range.  Both f16
matmul passes accumulate into the same PSUM bank (the common scale is
divided out with alpha afterwards), which yields fp32-class accuracy
(~3e-7 max rel) while each f16 matmul runs at full PE rate (~216 ns per
128x128x512 MM, weight loads hidden by FWL).  alpha/bias are applied on
the output tile with two DVE ops against partition-broadcast tiles.
"""

import numpy as np

import concourse.bass as bass
import concourse.tile as tile
from concourse import bacc
import concourse.mybir as mybir
from concourse.bass_utils import run_bass_kernel_spmd

F32 = mybir.dt.float32
F32R = mybir.dt.float32r
F16 = mybir.dt.float16
ALU = mybir.AluOpType

B, IN, OUT = 8192, 4096, 4096
NCORES = 8
OSH = OUT // NCORES          # 512 out_features per core
KT = IN // 128               # 32 contraction tiles
BT = B // 128                # 64 batch tiles per core
SC = 4096.0                  # lo-term scale for the f16 mode

MODE = "f16"                 # "f16" | "r2" | "r1"

_CACHE = {}


def _build(mode=MODE):
    wdt = F16 if mode == "f16" else F32R
    nc = bacc.Bacc("TRN2", target_bir_lowering=False, debug=False)
    # x pre-tiled on host: xT[bt, p, it, b] = x[bt*128 + b, it*128 + p]
    xt_d = nc.dram_tensor("xT", [BT, 128, KT, 128], F32, kind="ExternalInput").ap()
    wT_d = nc.dram_tensor("wT", [IN, OSH], F32, kind="ExternalInput").ap()
    alpha_d = nc.dram_tensor("alpha", [OSH], F32, kind="ExternalInput").ap()
    bias_d = nc.dram_tensor("bias", [OSH], F32, kind="ExternalInput").ap()
    out_d = nc.dram_tensor("out", [B, OSH], F32, kind="ExternalOutput").ap()

    with tile.TileContext(nc) as tc:
        with (
            tc.tile_pool(name="const", bufs=1) as const,
            tc.tile_pool(name="wstage", bufs=3) as wstage,
            tc.tile_pool(name="xpool", bufs=2) as xpool,
            tc.tile_pool(name="hpool", bufs=6) as hpool,
            tc.tile_pool(name="lpool", bufs=6) as lpool,
            tc.tile_pool(name="opool", bufs=4) as opool,
            tc.tile_pool(name="ps", bufs=8, space="PSUM") as ps,
        ):
            def load_chunk(bt):
                x_f = xpool.tile([128, KT, 128], F32, tag="x_f", name="x_f")
                nc.sync.dma_start(x_f[:], xt_d[bt])
                x_h = hpool.tile([128, KT, 128], wdt, tag="x_h", name="x_h")
                if mode == "f16":
                    # hi = f16(SC*x) (power-of-two scale, exact)
                    nc.scalar.mul(x_h[:], x_f[:], SC)
                else:
                    nc.scalar.copy(x_h[:], x_f[:])
                x_l = None
                if mode != "r1":
                    x_l = lpool.tile([128, KT, 128], wdt, tag="x_l",
                                     name="x_l")
                    if mode == "f16":
                        # lo = f16(SC*x - hi)
                        nc.vector.scalar_tensor_tensor(
                            x_l[:], x_f[:], SC, x_h[:],
                            ALU.mult, ALU.subtract)
                    else:
                        nc.vector.scalar_tensor_tensor(
                            x_l[:], x_f[:], 0.0, x_h[:],
                            ALU.bypass, ALU.subtract)
                return x_h, x_l

            # batch tiles processed in groups of G with the contraction loop
            # outermost: each weight k-tile feeds 2*G matmuls the moment it
            # arrives, so the W DMA stream never starves the PE during ramp-in
            G = 3
            groups = [list(range(g, min(g + G, BT))) for g in range(0, BT, G)]
            chunks = {}
            # group-0 x chunks interleaved with the W stream on the DMA queue
            chunks[groups[0][0]] = load_chunk(groups[0][0])

            # resident binarized weight shard, one tile per k-tile
            wT_t = wT_d.rearrange("(it p) o -> p it o", p=128)
            w_r = []
            for it in range(KT):
                if it == 8 and len(groups[0]) > 1:
                    chunks[groups[0][1]] = load_chunk(groups[0][1])
                if it == 16 and len(groups[0]) > 2:
                    chunks[groups[0][2]] = load_chunk(groups[0][2])
                w_f = wstage.tile([128, OSH], F32, tag="w_f", name="w_f")
                nc.sync.dma_start(w_f[:], wT_t[:, it, :])
                w_rt = const.tile([128, OSH], wdt, name=f"w_r{it}")
                nc.scalar.sign(w_rt[:], w_f[:])
                w_r.append(w_rt)

            alpha_b = const.tile([128, OSH], F32, name="alpha_b")
            nc.sync.dma_start(alpha_b[:], alpha_d.partition_broadcast(128))
            bias_b = const.tile([128, OSH], F32, name="bias_b")
            nc.sync.dma_start(bias_b[:], bias_d.partition_broadcast(128))
            if mode == "f16":
                alpha_eff = const.tile([128, OSH], F32, name="alpha_eff")
                nc.vector.tensor_scalar_mul(alpha_eff[:], alpha_b[:], 1.0 / SC)
            else:
                alpha_eff = alpha_b

            for gi, grp in enumerate(groups):
                pt = {b: ps.tile([128, OSH], F32, tag="p", name=f"p{b}")
                      for b in grp}
                nxt = groups[gi + 1] if gi + 1 < len(groups) else []
                load_at = {(j + 1) * KT // (len(nxt) + 1): nxt[j]
                           for j in range(len(nxt))}
                for it in range(KT):
                    if it in load_at:
                        chunks[load_at[it]] = load_chunk(load_at[it])
                    for b in grp:
                        x_h, x_l = chunks[b]
                        nc.tensor.matmul(
                            pt[b][:], x_h[:, it, :], w_r[it][:],
                            start=(it == 0),
                            stop=(mode == "r1" and it == KT - 1))
                        if mode != "r1":
                            nc.tensor.matmul(
                                pt[b][:], x_l[:, it, :], w_r[it][:],
                                start=False, stop=(it == KT - 1))
                for b in grp:
                    del chunks[b]
                    # out = p * alpha_eff + bias (alpha_eff = alpha/SC for f16)
                    t = opool.tile([128, OSH], F32, tag="t", name="t")
                    nc.vector.scalar_tensor_tensor(
                        t[:], pt[b][:], 0.0, alpha_eff[:],
                        ALU.bypass, ALU.mult)
                    o = opool.tile([128, OSH], F32, tag="o", name="o")
                    nc.vector.tensor_add(o[:], t[:], bias_b[:])
                    nc.sync.dma_start(out_d[bass.ts(b, 128), :], o[:])

    nc.compile()
    return nc


def _prep_inputs(x, weight_fp, alpha, bias):
    x = np.asarray(x, dtype=np.float32)
    weight_fp = np.asarray(weight_fp, dtype=np.float32)
    alpha = np.asarray(alpha, dtype=np.float32).reshape(-1)
    bias = np.asarray(bias, dtype=np.float32).reshape(-1)
    assert x.shape == (B, IN) and weight_fp.shape == (OUT, IN)

    # [bt, p, it, b] <- x[bt*128+b, it*128+p]
    xT = np.ascontiguousarray(
        x.reshape(BT, 128, KT, 128).transpose(0, 3, 2, 1)
    )
    in_maps = []
    for c in range(NCORES):
        sl = slice(c * OSH, (c + 1) * OSH)
        in_maps.append({
            "xT": xT,
            "wT": np.ascontiguousarray(weight_fp[sl].T),
            "alpha": np.ascontiguousarray(alpha[sl]),
            "bias": np.ascontiguousarray(bias[sl]),
        })
    return in_maps


def kernel(x, weight_fp, alpha, bias):
    if "nc" not in _CACHE:
        _CACHE["nc"] = _build()
    nc = _CACHE["nc"]
    in_maps = _prep_inputs(x, weight_fp, alpha, bias)
    res = run_bass_kernel_spmd(nc, in_maps, list(range(NCORES)))
    out = np.concatenate(
        [res.results[c]["out"] for c in range(NCORES)], axis=1
    )
    return np.ascontiguousarray(out, dtype=np.float32)
